# revision 3
# baseline (speedup 1.0000x reference)
"""MoE (top-2 of 8 experts) Trainium2 Bass kernel.

Distribution: data-parallel — each of the 8 NeuronCores processes one batch row
(4096 tokens) with replicated weights. No collectives.

Per-core pipeline:
  1. gating: fp32 logits = x @ gate_w.T on the PE (tokens on partitions)
  2. routing: top-2 via vector.max + is_equal onehots; softmax-of-2 == sigmoid;
     rank = cross-partition exclusive prefix (strict-upper-triangular matmul)
          + free-axis prefix over token tiles (tensor_tensor_scan);
     position = expert*C + rank (capacity C=1536, max observed load 1124).
  3. dispatch: dma_scatter_add copies each token's bf16 x row to its two
     positions in xg[EC, D] (pre-zeroed; positions are distinct so += = write).
     Index lists are int16, wrapped [j%16, j//16] and replicated to all 8
     Q7 core groups.
  4. expert FFN over xg, bf16 matmuls with fp32 PSUM accumulate:
     xgT tiles via HWDGE transpose-DMA loads (feature-major),
     hT[h,tok] = silu(w1t.T@xgT) * (w3t.T@xgT),
     y[tok,d] = hT.T @ w2t  (hT chunks stationary) -> yg rows (token-major).
  5. combine: per token tile, indirect-gather y rows at pos0/pos1 (one offset
     per partition), out = w0*y0 + w1*y1, stored fp32.
"""
from contextlib import ExitStack

import numpy as np
import ml_dtypes

import concourse.bass as bass
import concourse.bacc as bacc
import concourse.tile as tile
import concourse.mybir as mybir
from concourse.masks import make_upper_triangular
from concourse.bass_utils import run_bass_kernel_spmd

dt = mybir.dt
AF = mybir.ActivationFunctionType
ALU = mybir.AluOpType

P = 128
N = 4096           # tokens per core
NT = N // P        # 32 token tiles
D = 512
DC = D // P        # 4 d-chunks
H = 1536
HC = H // P        # 12 h-chunks
E = 8
C = 1536           # capacity per expert (multiple of 512)
GPE = C // 512     # groups per expert
EC = E * C
TG = N // 512      # 8 token groups
NCORES = 8


def build_moe_nc(silu_native: bool = True):
    nc = bacc.Bacc("TRN2", target_bir_lowering=False, debug=False)

    xT = nc.dram_tensor("xT", [D, N], dt.float32, kind="ExternalInput").ap()
    xb = nc.dram_tensor("xb", [N, D], dt.bfloat16, kind="ExternalInput").ap()
    gwT = nc.dram_tensor("gwT", [D, E], dt.float32, kind="ExternalInput").ap()
    w1t = nc.dram_tensor("w1t", [E, D, H], dt.bfloat16, kind="ExternalInput").ap()
    w3t = nc.dram_tensor("w3t", [E, D, H], dt.bfloat16, kind="ExternalInput").ap()
    w2t = nc.dram_tensor("w2t", [E, H, D], dt.bfloat16, kind="ExternalInput").ap()
    out = nc.dram_tensor("out", [N, D], dt.float32, kind="ExternalOutput").ap()

    posbuf = nc.dram_tensor("posbuf", [2, N], dt.int16).ap()
    xg = nc.dram_tensor("xg", [EC, D], dt.bfloat16).ap()
    yg = nc.dram_tensor("yg", [EC, D], dt.float32).ap()

    with tile.TileContext(nc) as tc, ExitStack() as ctx:
        consts = ctx.enter_context(tc.tile_pool(name="consts", bufs=1))
        route = ctx.enter_context(tc.tile_pool(name="route", bufs=1))
        loop = ctx.enter_context(tc.tile_pool(name="loop", bufs=3))

        # ---- constants ----
        Lexcl = consts.tile([P, P], dt.float32)
        make_upper_triangular(nc, Lexcl[:], val=1.0, diag=False)
        iota8_i = consts.tile([P, E], dt.int32)
        nc.gpsimd.iota(iota8_i[:], pattern=[[1, E]], base=0, channel_multiplier=0)
        cbase = consts.tile([P, E], dt.float32)
        nc.vector.tensor_copy(cbase[:], iota8_i[:])
        nc.vector.tensor_scalar_mul(cbase[:], cbase[:], float(C))
        gwT_sb = consts.tile([P, DC, E], dt.float32)
        nc.sync.dma_start(out=gwT_sb[:], in_=gwT.rearrange("(c p) e -> p c e", p=P))

        # zero xg early (padding positions must be finite)
        zng = consts.tile([P, 4096], dt.bfloat16)
        nc.vector.memset(zng[:], 0)
        xgz = xg.rearrange("(a p) d -> p a d", p=P)       # [128, 96, 512]
        for i in range(12):
            nc.sync.dma_start(out=xgz[:, i * 8:(i + 1) * 8, :],
                              in_=zng[:].rearrange("p (a d) -> p a d", d=D))

        # ---- routing arrays ----
        oh0 = route.tile([P, E, NT], dt.float32)
        oh1 = route.tile([P, E, NT], dt.float32)
        cnt = route.tile([P, E, NT], dt.float32)
        wts = route.tile([P, 2, NT], dt.float32)

        # ---- phase 1+2a: gating per token tile ----
        rctx = ExitStack()
        pgs = rctx.enter_context(tc.tile_pool(name="pgs", bufs=2, space="PSUM"))
        with tc.tile_pool(name="xt", bufs=1) as xtp:
            xT_sb = xtp.tile([P, DC, N], dt.float32)
            nc.sync.dma_start(out=xT_sb[:], in_=xT.rearrange("(c p) n -> p c n", p=P))

            for T in range(NT):
                ts = slice(T * P, (T + 1) * P)
                pg = pgs.tile([P, E], dt.float32, space="PSUM", tag="pg")
                for c in range(DC):
                    nc.tensor.matmul(pg[:], lhsT=xT_sb[:, c, ts], rhs=gwT_sb[:, c, :],
                                     start=(c == 0), stop=(c == DC - 1))
                lg = loop.tile([P, E], dt.float32, tag="lg")
                nc.vector.tensor_copy(lg[:], pg[:])
                mx = loop.tile([P, E], dt.float32, tag="mx")
                nc.vector.max(out=mx[:], in_=lg[:])
                diff = loop.tile([P, 1], dt.float32, tag="diff")
                nc.vector.tensor_sub(diff[:], mx[:, 0:1], mx[:, 1:2])
                nc.scalar.activation(wts[:, 0, T:T + 1], diff[:], AF.Sigmoid)
                nc.vector.tensor_scalar(wts[:, 1, T:T + 1], wts[:, 0, T:T + 1], -1.0, 1.0,
                                        op0=ALU.mult, op1=ALU.add)
                nc.vector.tensor_tensor(out=oh0[:, :, T], in0=lg[:],
                                        in1=mx[:, 0:1].to_broadcast([P, E]), op=ALU.is_equal)
                nc.vector.tensor_tensor(out=oh1[:, :, T], in0=lg[:],
                                        in1=mx[:, 1:2].to_broadcast([P, E]), op=ALU.is_equal)
                nc.vector.tensor_add(cnt[:, :, T], oh0[:, :, T], oh1[:, :, T])

        # ---- phase 2b: ranking / positions ----
        rowtot = route.tile([P, E], dt.float32)
        nc.vector.reduce_sum(out=rowtot[:], in_=cnt[:], axis=mybir.AxisListType.X)
        partpre = pgs.tile([P, E], dt.float32, space="PSUM", tag="partpre")
        nc.tensor.matmul(partpre[:], lhsT=Lexcl[:], rhs=rowtot[:], start=True, stop=True)

        posmat = route.tile([P, E, NT], dt.float32)
        for e in range(E):
            nc.vector.tensor_tensor_scan(out=posmat[:, e, :], data0=cnt[:, e, :],
                                         data1=cnt[:, e, :], initial=0.0,
                                         op0=ALU.add, op1=ALU.bypass)
        nc.vector.tensor_sub(posmat[:], posmat[:], cnt[:])
        base_e = route.tile([P, E], dt.float32)
        nc.vector.tensor_add(base_e[:], cbase[:], partpre[:])
        nc.vector.tensor_tensor(out=posmat[:], in0=posmat[:],
                                in1=base_e[:, :, None].to_broadcast([P, E, NT]), op=ALU.add)

        def select_e(dst, oh):
            prod = loop.tile([P, E, NT], dt.float32, tag="prod")
            nc.vector.tensor_mul(prod[:], oh[:], posmat[:])
            r4 = loop.tile([P, 4, NT], dt.float32, tag="r4")
            nc.vector.tensor_add(r4[:], prod[:, 0:4, :], prod[:, 4:8, :])
            r2 = loop.tile([P, 2, NT], dt.float32, tag="r2")
            nc.vector.tensor_add(r2[:], r4[:, 0:2, :], r4[:, 2:4, :])
            nc.vector.tensor_add(dst, r2[:, 0, :], r2[:, 1, :])

        pos_f = route.tile([P, 2, NT], dt.float32)
        select_e(pos_f[:, 0, :], oh0)
        select_e(pos_f[:, 1, :], oh1)
        pos_i = route.tile([P, 2, NT], dt.int32)
        nc.vector.tensor_copy(pos_i[:], pos_f[:])
        pos_16 = route.tile([P, 2, NT], dt.int16)
        nc.vector.tensor_copy(pos_16[:], pos_f[:])

        # positions j-order -> DRAM, then wrapped+replicated reload
        nc.sync.dma_start(
            out=posbuf.rearrange("s (t p) -> p s t", p=P), in_=pos_16[:])
        posw = route.tile([P, 2, N // 16], dt.int16)
        for g in range(8):
            nc.sync.dma_start(out=posw[16 * g:16 * (g + 1), :, :],
                              in_=posbuf.rearrange("s (t pp) -> pp s t", pp=16))

        rctx.close()

        # ---- phase 3a: dispatch x rows to positions ----
        dpool = ctx.enter_context(tc.tile_pool(name="dpool", bufs=3))
        for tg in range(TG):
            xrow = dpool.tile([P, 4, 512], dt.bfloat16, tag="xrow")
            nc.sync.dma_start(out=xrow[:],
                              in_=xb[tg * 512:(tg + 1) * 512, :].rearrange("(c p) d -> p c d", p=P))
            for k in range(2):
                nc.gpsimd.dma_scatter_add(
                    out_ap=xg, in_ap=xrow[:],
                    idxs_ap=posw[:, k, tg * 32:(tg + 1) * 32],
                    num_idxs=512, num_idxs_reg=512, elem_size=D)

        # ---- phase 3b: expert FFN ----
        wpool = ctx.enter_context(tc.tile_pool(name="wpool", bufs=2))
        fpool = ctx.enter_context(tc.tile_pool(name="fpool", bufs=2))
        spool = ctx.enter_context(tc.tile_pool(name="spool", bufs=3))
        pAB = ctx.enter_context(tc.tile_pool(name="pAB", bufs=2, space="PSUM"))
        pY = ctx.enter_context(tc.tile_pool(name="pY", bufs=2, space="PSUM"))

        for e in range(E):
            w1_sb = wpool.tile([P, DC, H], dt.bfloat16, tag="w1")
            w3_sb = wpool.tile([P, DC, H], dt.bfloat16, tag="w3")
            w2_sb = wpool.tile([P, HC, D], dt.bfloat16, tag="w2")
            nc.sync.dma_start(out=w1_sb[:], in_=w1t[e].rearrange("(c p) h -> p c h", p=P))
            nc.sync.dma_start(out=w3_sb[:], in_=w3t[e].rearrange("(c p) h -> p c h", p=P))
            nc.sync.dma_start(out=w2_sb[:], in_=w2t[e].rearrange("(c p) d -> p c d", p=P))

            for g in range(GPE):
                gpos = e * GPE + g      # global 512-position group
                r0 = gpos * 512
                xgT = fpool.tile([P, DC, 512], dt.bfloat16, tag="xgT")
                for c in range(DC):
                    nc.sync.dma_start(out=xgT[:, c, :],
                                      in_=xg[r0:r0 + 512, c * P:(c + 1) * P],
                                      transpose=True)

                hT = fpool.tile([P, HC, 512], dt.bfloat16, tag="hT")
                for hc in range(HC):
                    hs = slice(hc * P, (hc + 1) * P)
                    pA = pAB.tile([P, 512], dt.float32, space="PSUM", tag="pA")
                    pB = pAB.tile([P, 512], dt.float32, space="PSUM", tag="pB")
                    for c in range(DC):
                        nc.tensor.matmul(pA[:], lhsT=w1_sb[:, c, hs], rhs=xgT[:, c, :],
                                         start=(c == 0), stop=(c == DC - 1))
                    for c in range(DC):
                        nc.tensor.matmul(pB[:], lhsT=w3_sb[:, c, hs], rhs=xgT[:, c, :],
                                         start=(c == 0), stop=(c == DC - 1))
                    st = spool.tile([P, 512], dt.bfloat16, tag="silu")
                    if silu_native:
                        nc.scalar.activation(st[:], pA[:], AF.Silu)
                    else:  # CoreSim lacks Silu: sigmoid + explicit mult
                        sg = spool.tile([P, 512], dt.float32, tag="sg")
                        nc.scalar.activation(sg[:], pA[:], AF.Sigmoid)
                        nc.vector.tensor_mul(st[:], sg[:], pA[:])
                    nc.vector.tensor_mul(hT[:, hc, :], st[:], pB[:])

                for tc_ in range(4):
                    ps = slice(tc_ * P, (tc_ + 1) * P)
                    py = pY.tile([P, D], dt.float32, space="PSUM", tag="py")
                    for hc in range(HC):
                        nc.tensor.matmul(py[:], lhsT=hT[:, hc, ps], rhs=w2_sb[:, hc, :],
                                         start=(hc == 0), stop=(hc == HC - 1))
                    y_sb = spool.tile([P, D], dt.float32, tag="ysb")
                    nc.vector.tensor_copy(y_sb[:], py[:])
                    nc.sync.dma_start(out=yg[r0 + tc_ * P:r0 + (tc_ + 1) * P, :], in_=y_sb[:])

        # ---- phase 4: combine ----
        cpool = ctx.enter_context(tc.tile_pool(name="cpool", bufs=4))
        for T in range(NT):
            yg0 = cpool.tile([P, D], dt.float32, tag="yg0")
            yg1 = cpool.tile([P, D], dt.float32, tag="yg1")
            nc.gpsimd.indirect_dma_start(
                out=yg0[:], out_offset=None, in_=yg[:, :],
                in_offset=bass.IndirectOffsetOnAxis(ap=pos_i[:, 0, T:T + 1], axis=0))
            nc.gpsimd.indirect_dma_start(
                out=yg1[:], out_offset=None, in_=yg[:, :],
                in_offset=bass.IndirectOffsetOnAxis(ap=pos_i[:, 1, T:T + 1], axis=0))
            o_sb = cpool.tile([P, D], dt.float32, tag="osb")
            nc.vector.tensor_scalar_mul(o_sb[:], yg0[:], wts[:, 0, T:T + 1])
            nc.vector.scalar_tensor_tensor(out=o_sb[:], in0=yg1[:],
                                           scalar=wts[:, 1, T:T + 1], in1=o_sb[:],
                                           op0=ALU.mult, op1=ALU.add)
            nc.sync.dma_start(out=out[T * P:(T + 1) * P, :], in_=o_sb[:])

    nc.compile()
    return nc


_CACHE = {}


def _prep_maps(x, gate_w, w1, w3, w2):
    bf16 = ml_dtypes.bfloat16
    gwT = np.ascontiguousarray(gate_w.T).astype(np.float32)
    w1t = np.ascontiguousarray(w1.transpose(0, 2, 1)).astype(bf16)
    w3t = np.ascontiguousarray(w3.transpose(0, 2, 1)).astype(bf16)
    w2t = np.ascontiguousarray(w2.transpose(0, 2, 1)).astype(bf16)
    in_maps = []
    for c in range(NCORES):
        xc = np.asarray(x[c], dtype=np.float32)
        in_maps.append({
            "xT": np.ascontiguousarray(xc.T),
            "xb": np.ascontiguousarray(xc).astype(bf16),
            "gwT": gwT, "w1t": w1t, "w3t": w3t, "w2t": w2t,
        })
    return in_maps


def kernel(x, gate_w, w1, w3, w2, _trace=False, **trace_kwargs):
    x = np.asarray(x, dtype=np.float32)
    gate_w = np.asarray(gate_w, dtype=np.float32)
    w1 = np.asarray(w1, dtype=np.float32)
    w3 = np.asarray(w3, dtype=np.float32)
    w2 = np.asarray(w2, dtype=np.float32)
    assert x.shape == (NCORES, N, D), x.shape

    if "nc" not in _CACHE:
        _CACHE["nc"] = build_moe_nc()
    nc = _CACHE["nc"]

    in_maps = _prep_maps(x, gate_w, w1, w3, w2)
    res = run_bass_kernel_spmd(nc, in_maps, core_ids=list(range(NCORES)),
                               trace=_trace, **trace_kwargs)
    out = np.stack([res.results[c]["out"] for c in range(NCORES)], axis=0)
    if _trace:
        _CACHE["last_result"] = res
    return out
